# revision 7
# baseline (speedup 1.0000x reference)
"""Multi-head attention (B=2, H=16, S=2048, D=64) on 8 TRN2 NeuronCores.

Returns (out, p_attn) matching the reference:
    scores = QK^T/sqrt(D); masked (mask==0 -> -1e9); p_attn = softmax(scores)
    out = p_attn @ V

Sharding: batch x heads across 8 cores -> 4 heads per core, no cross-core
communication. Core c handles batch c//4, heads (c%4)*4 .. +4.

Per-core device algorithm, all in the TRANSPOSED [k, q] layout so the PV
matmul needs no on-chip transposes (the host view-transposes the outputs):
  S^T[k,q] = K^T.T @ Q^T          (f32r matmuls, contraction d=64)
           + I.T @ A^T            (fp8e5 matmul adds additive mask A=(m-1)*4096;
                                   any A <= -840 underflows exp to exactly 0)
  Pu^T     = exp(S^T / 8)         (ACT, PSUM->SBUF; masked entries -> 0.0)
  outT'    = [V | 1].T @ Pu^T     (f32r matmul; row 64 = softmax denominator)
  r        = exp(-ln(denom))      (ACT reciprocal, same table set as exp)
  rb       = ones.T @ r           (K=1 fp32 matmul broadcasts r across partitions)
  p_attn^T = Pu^T * rb            (DVE in-place, step-0 free-dim repeat of rb)
  outT     = outT'[0:64] * rb[0:64]
"""

import sys

sys.path.insert(0, "/opt/trn_rl_repo")

import numpy as np
import ml_dtypes

import concourse.bass as bass
import concourse.bacc as bacc
import concourse.mybir as mybir
import concourse.tile as tile
from concourse.bass_utils import run_bass_kernel_spmd

F32 = mybir.dt.float32
F32R = mybir.dt.float32r
FP8 = mybir.dt.float8e5
AF = mybir.ActivationFunctionType

B, H, S, D = 2, 16, 2048, 64
NCORES = 8
HPC = 4          # heads per core
QCHUNK = 512     # q processed per inner block
NQC = S // QCHUNK
NKT = S // 128   # 16 k-tiles of 128
AMASK = -4096.0  # additive mask value (exp((-4096 + s)/8) == 0.0 in fp32)


_ORIG_ACT_TABLES = bacc.get_activation_tables


def _patched_act_tables(arch):
    """Blank the exp-only table sets so every Exp resolves to the set that
    also contains Ln -> a single ACT table load instead of per-call thrash.
    Positions are preserved (index == act_func_set_id)."""
    tabs = _ORIG_ACT_TABLES(arch)
    out = {}
    for name, funcs in tabs.items():
        if name in ("exp_and_others", "exp_and_friends"):
            out[name] = set()
        else:
            out[name] = funcs
    return out


def build(hpc=HPC, nqc=NQC):
    bacc.get_activation_tables = _patched_act_tables
    try:
        return _build(hpc, nqc)
    finally:
        bacc.get_activation_tables = _ORIG_ACT_TABLES


def _build(hpc, nqc):
    nc = bacc.Bacc("TRN2", target_bir_lowering=False, debug=False)

    qT_d = nc.dram_tensor("qT", [hpc // 2, 128, S], F32R, kind="ExternalInput")
    kT_d = nc.dram_tensor("kT", [hpc // 2, 128, S], F32R, kind="ExternalInput")
    # v pre-rearranged on host to [h, partition, ktile, d+1] with ones column
    v_d = nc.dram_tensor("v", [hpc, 128, NKT, D + 1], F32R, kind="ExternalInput")
    mA_d = nc.dram_tensor("mA", [128, NQC, NKT, QCHUNK], FP8, kind="ExternalInput")
    id_d = nc.dram_tensor("ident", [128, 128], FP8, kind="ExternalInput")

    pT_d = nc.dram_tensor("pT", [hpc, S, S], F32R, kind="ExternalOutput")
    outT_d = nc.dram_tensor("outT", [hpc, D, S], F32, kind="ExternalOutput")

    with tile.TileContext(nc) as tc:
        with (
            tc.tile_pool(name="const", bufs=1) as const,
            tc.tile_pool(name="amask", bufs=2) as a_pool,
            tc.tile_pool(name="pu", bufs=3) as pu_pool,
            tc.tile_pool(name="ot", bufs=1) as ot_pool,
            tc.tile_pool(name="small", bufs=2) as small,
            tc.tile_pool(name="spsum", bufs=2, space="PSUM") as s_pool,
            tc.tile_pool(name="pvpsum", bufs=2, space="PSUM") as pv_pool,
            tc.tile_pool(name="bcpsum", bufs=2, space="PSUM") as bc_pool,
        ):
            # ---- constants, loaded once ----
            qT_sb = [const.tile([128, S], F32R, tag=f"qT{p}", name=f"qT{p}")
                     for p in range(hpc // 2)]
            kT_sb = [const.tile([128, S], F32R, tag=f"kT{p}", name=f"kT{p}")
                     for p in range(hpc // 2)]
            v_sb = [const.tile([128, NKT, D + 1], F32R, tag=f"v{h}", name=f"v{h}")
                    for h in range(hpc)]
            id_sb = const.tile([128, 128], FP8, tag="ident")
            ones_sb = const.tile([128, 128], F32, tag="ones")

            for p in range(hpc // 2):
                nc.gpsimd.dma_start(qT_sb[p][:], qT_d[p])
                nc.gpsimd.dma_start(kT_sb[p][:], kT_d[p])
            for h in range(hpc):
                nc.gpsimd.dma_start(v_sb[h][:], v_d[h])
            nc.gpsimd.dma_start(id_sb[:], id_d[:])
            nc.vector.memset(ones_sb[:], 1.0)

            pT_re = [pT_d[h].rearrange("(t p) q -> p t q", p=128) for h in range(hpc)]

            for qc in range(nqc):
                qs = slice(qc * QCHUNK, (qc + 1) * QCHUNK)
                a_sb = a_pool.tile([128, NKT, QCHUNK], FP8, tag="a")
                nc.gpsimd.dma_start(a_sb[:], mA_d[:, qc, :, :])
                osb = ot_pool.tile([D, hpc, QCHUNK], F32, tag="osb")
                for h in range(hpc):
                    hp, hi = divmod(h, 2)
                    prow = slice(hi * 64, (hi + 1) * 64)
                    # ---- S^T = K^T.T @ Q^T + I.T @ A^T ; Pu = exp(S^T/8) ----
                    pu = pu_pool.tile([128, NKT, QCHUNK], F32R, tag="pu")
                    for ktp in range(NKT // 2):
                        sp = s_pool.tile([128, 2 * QCHUNK], F32, tag="s")
                        for j in range(2):
                            kt = 2 * ktp + j
                            cs = slice(j * QCHUNK, (j + 1) * QCHUNK)
                            nc.tensor.matmul(
                                sp[:, cs],
                                kT_sb[hp][prow, kt * 128 : (kt + 1) * 128],
                                qT_sb[hp][prow, qs],
                                start=True,
                                stop=False,
                            )
                            nc.tensor.matmul(
                                sp[:, cs], id_sb[:], a_sb[:, kt, :],
                                start=False, stop=True,
                            )
                        nc.scalar.activation(
                            pu[:, 2 * ktp : 2 * ktp + 2, :], sp[:], AF.Exp,
                            scale=0.125,
                        )
                    # ---- outT' = [V|1].T @ Pu ----
                    pv = pv_pool.tile([D + 1, QCHUNK], F32, tag="pv")
                    for kt in range(NKT):
                        nc.tensor.matmul(
                            pv[:], v_sb[h][:, kt, :], pu[:, kt, :],
                            start=(kt == 0), stop=(kt == NKT - 1),
                        )
                    # ---- r = exp(-ln(denom)); rb = broadcast(r) ----
                    lnd = small.tile([D + 1, QCHUNK], F32, tag="lnd")
                    nc.scalar.activation(lnd[D : D + 1, :], pv[D : D + 1, :], AF.Ln)
                    r1 = small.tile([D + 1, QCHUNK], F32, tag="r1")
                    nc.scalar.activation(
                        r1[D : D + 1, :], lnd[D : D + 1, :], AF.Exp, scale=-1.0
                    )
                    bc = bc_pool.tile([128, QCHUNK], F32, tag="bc")
                    nc.tensor.matmul(
                        bc[:], ones_sb[D : D + 1, :], r1[D : D + 1, :],
                        start=True, stop=True,
                    )
                    rb = small.tile([D, QCHUNK], F32, tag="rb")
                    nc.vector.tensor_copy(rb[:], bc[0:D, :])
                    # ---- outT column for this head ----
                    nc.vector.tensor_mul(osb[:, h, :], pv[0:D, :], rb[:])
                    # ---- p_attn^T = Pu * rb, in place; store ----
                    bc_ap = bc[:]
                    bc_rep = bass.AP(
                        tensor=bc_ap.tensor,
                        offset=bc_ap.offset,
                        ap=[bc_ap.ap[0], [0, NKT], bc_ap.ap[1]],
                    )
                    nc.vector.tensor_mul(pu[:], pu[:], bc_rep)
                    nc.sync.dma_start(pT_re[h][:, :, qs], pu[:])
                # one batched outT store per q-chunk: [d, h, q] -> [h, d, q]
                o_ap = outT_d[:]
                o_dst = bass.AP(
                    tensor=o_ap.tensor,
                    offset=o_ap.offset + qc * QCHUNK,
                    ap=[[S, D], [D * S, hpc], [1, QCHUNK]],
                )
                nc.gpsimd.dma_start(o_dst, osb[:])

    nc.compile()
    return nc


_CACHE = {}


def _get_nc():
    if "nc" not in _CACHE:
        _CACHE["nc"] = build()
    return _CACHE["nc"]


def _make_in_maps(query, key, value, mask):
    ident = np.eye(128, dtype=np.float32).astype(ml_dtypes.float8_e5m2)
    in_maps = []
    for c in range(NCORES):
        b, h0 = divmod(c, NCORES // B)
        h0 *= HPC
        q4 = query[b, h0 : h0 + HPC]  # [4, S, D]
        k4 = key[b, h0 : h0 + HPC]
        v4 = np.concatenate(
            [
                np.asarray(value[b, h0 : h0 + HPC], dtype=np.float32),
                np.ones((HPC, S, 1), dtype=np.float32),
            ],
            axis=-1,
        )  # [4, S, 65]
        # [h, S, 65] -> [h, ktile, 128, 65] -> [h, 128, ktile, 65]
        v4 = np.ascontiguousarray(
            v4.reshape(HPC, NKT, 128, D + 1).transpose(0, 2, 1, 3)
        )
        qT = np.ascontiguousarray(q4.transpose(0, 2, 1)).reshape(HPC // 2, 128, S)
        kT = np.ascontiguousarray(k4.transpose(0, 2, 1)).reshape(HPC // 2, 128, S)
        mA = ((mask[b, 0].T.astype(np.float32) - 1.0) * (-AMASK)).astype(
            ml_dtypes.float8_e5m2
        )
        # [k, q] -> [p, qc, ktile, q'] so each partition's per-chunk slice is
        # 8KB contiguous in DRAM
        mA = mA.reshape(NKT, 128, NQC, QCHUNK).transpose(1, 2, 0, 3)
        in_maps.append(
            {"qT": qT, "kT": kT, "v": v4, "mA": np.ascontiguousarray(mA),
             "ident": ident}
        )
    return in_maps


def run_device(query, key, value, mask, trace=False, **kw):
    nc = _get_nc()
    in_maps = _make_in_maps(query, key, value, mask)
    return run_bass_kernel_spmd(nc, in_maps, list(range(NCORES)), trace=trace, **kw)


def kernel(query, key, value, mask):
    query = np.asarray(query, dtype=np.float32)
    key = np.asarray(key, dtype=np.float32)
    value = np.asarray(value, dtype=np.float32)
    mask = np.asarray(mask)
    res = run_device(query, key, value, mask)
    out = np.empty((B, H, S, D), dtype=np.float32)
    p_attn = np.empty((B, H, S, S), dtype=np.float32)
    for c, r in enumerate(res.results):
        b, h0 = divmod(c, NCORES // B)
        h0 *= HPC
        out[b, h0 : h0 + HPC] = r["outT"].transpose(0, 2, 1)
        p_attn[b, h0 : h0 + HPC] = r["pT"].transpose(0, 2, 1)
    return out, p_attn


# revision 8
# speedup vs baseline: 1.0008x; 1.0008x over previous
"""Multi-head attention (B=2, H=16, S=2048, D=64) on 8 TRN2 NeuronCores.

Returns (out, p_attn) matching the reference:
    scores = QK^T/sqrt(D); masked (mask==0 -> -1e9); p_attn = softmax(scores)
    out = p_attn @ V

Sharding: batch x heads across 8 cores -> 4 heads per core, no cross-core
communication. Core c handles batch c//4, heads (c%4)*4 .. +4.

Per-core device algorithm, all in the TRANSPOSED [k, q] layout so the PV
matmul needs no on-chip transposes (the host view-transposes the outputs):
  S^T[k,q] = K^T.T @ Q^T          (f32r matmuls, contraction d=64)
           + I.T @ A^T            (fp8e5 matmul adds additive mask A=(m-1)*4096;
                                   any A <= -840 underflows exp to exactly 0)
  Pu^T     = exp(S^T / 8)         (ACT, PSUM->SBUF; masked entries -> 0.0)
  outT'    = [V | 1].T @ Pu^T     (f32r matmul; row 64 = softmax denominator)
  r        = exp(-ln(denom))      (ACT reciprocal, same table set as exp)
  rb       = ones.T @ r           (K=1 fp32 matmul broadcasts r across partitions)
  p_attn^T = Pu^T * rb            (DVE in-place, step-0 free-dim repeat of rb)
  outT     = outT'[0:64] * rb[0:64]
"""

import sys

sys.path.insert(0, "/opt/trn_rl_repo")

import numpy as np
import ml_dtypes

import concourse.bass as bass
import concourse.bacc as bacc
import concourse.mybir as mybir
import concourse.tile as tile
from concourse.bass_utils import run_bass_kernel_spmd

F32 = mybir.dt.float32
F32R = mybir.dt.float32r
FP8 = mybir.dt.float8e5
AF = mybir.ActivationFunctionType

B, H, S, D = 2, 16, 2048, 64
NCORES = 8
HPC = 4          # heads per core
QCHUNK = 512     # q processed per inner block
NQC = S // QCHUNK
NKT = S // 128   # 16 k-tiles of 128
AMASK = -4096.0  # additive mask value (exp((-4096 + s)/8) == 0.0 in fp32)


_ORIG_ACT_TABLES = bacc.get_activation_tables


def _patched_act_tables(arch):
    """Blank the exp-only table sets so every Exp resolves to the set that
    also contains Ln -> a single ACT table load instead of per-call thrash.
    Positions are preserved (index == act_func_set_id)."""
    tabs = _ORIG_ACT_TABLES(arch)
    out = {}
    for name, funcs in tabs.items():
        if name in ("exp_and_others", "exp_and_friends"):
            out[name] = set()
        else:
            out[name] = funcs
    return out


def build(hpc=HPC, nqc=NQC):
    bacc.get_activation_tables = _patched_act_tables
    try:
        return _build(hpc, nqc)
    finally:
        bacc.get_activation_tables = _ORIG_ACT_TABLES


def _build(hpc, nqc):
    nc = bacc.Bacc("TRN2", target_bir_lowering=False, debug=False)

    qT_d = nc.dram_tensor("qT", [hpc // 2, 128, S], F32R, kind="ExternalInput")
    kT_d = nc.dram_tensor("kT", [hpc // 2, 128, S], F32R, kind="ExternalInput")
    # v pre-rearranged on host to [h, partition, ktile, d+1] with ones column
    v_d = nc.dram_tensor("v", [hpc, 128, NKT, D + 1], F32R, kind="ExternalInput")
    mA_d = nc.dram_tensor("mA", [128, NQC, NKT, QCHUNK], FP8, kind="ExternalInput")
    id_d = nc.dram_tensor("ident", [128, 128], FP8, kind="ExternalInput")

    pT_d = nc.dram_tensor("pT", [hpc, S, S], F32R, kind="ExternalOutput")
    outT_d = nc.dram_tensor("outT", [hpc, D, S], F32, kind="ExternalOutput")

    with tile.TileContext(nc) as tc:
        with (
            tc.tile_pool(name="const", bufs=1) as const,
            tc.tile_pool(name="amask", bufs=2) as a_pool,
            tc.tile_pool(name="pu", bufs=3) as pu_pool,
            tc.tile_pool(name="ot", bufs=1) as ot_pool,
            tc.tile_pool(name="small", bufs=2) as small,
            tc.tile_pool(name="spsum", bufs=2, space="PSUM") as s_pool,
            tc.tile_pool(name="pvpsum", bufs=2, space="PSUM") as pv_pool,
            tc.tile_pool(name="bcpsum", bufs=2, space="PSUM") as bc_pool,
        ):
            # ---- constants, loaded once ----
            qT_sb = [const.tile([128, S], F32R, tag=f"qT{p}", name=f"qT{p}")
                     for p in range(hpc // 2)]
            kT_sb = [const.tile([128, S], F32R, tag=f"kT{p}", name=f"kT{p}")
                     for p in range(hpc // 2)]
            v_sb = [const.tile([128, NKT, D + 1], F32R, tag=f"v{h}", name=f"v{h}")
                    for h in range(hpc)]
            id_sb = const.tile([128, 128], FP8, tag="ident")
            ones_sb = const.tile([128, 128], F32, tag="ones")

            for p in range(hpc // 2):
                nc.gpsimd.dma_start(qT_sb[p][:], qT_d[p])
                nc.gpsimd.dma_start(kT_sb[p][:], kT_d[p])
            for h in range(hpc):
                nc.sync.dma_start(v_sb[h][:], v_d[h])
            nc.gpsimd.dma_start(id_sb[:], id_d[:])
            nc.vector.memset(ones_sb[:], 1.0)

            pT_re = [pT_d[h].rearrange("(t p) q -> p t q", p=128) for h in range(hpc)]

            for qc in range(nqc):
                qs = slice(qc * QCHUNK, (qc + 1) * QCHUNK)
                a_sb = a_pool.tile([128, NKT, QCHUNK], FP8, tag="a")
                nc.gpsimd.dma_start(a_sb[:], mA_d[:, qc, :, :])
                osb = ot_pool.tile([D, hpc, QCHUNK], F32, tag="osb")
                for h in range(hpc):
                    hp, hi = divmod(h, 2)
                    prow = slice(hi * 64, (hi + 1) * 64)
                    # ---- S^T = K^T.T @ Q^T + I.T @ A^T ; Pu = exp(S^T/8) ----
                    pu = pu_pool.tile([128, NKT, QCHUNK], F32R, tag="pu")
                    for ktp in range(NKT // 2):
                        sp = s_pool.tile([128, 2 * QCHUNK], F32, tag="s")
                        for j in range(2):
                            kt = 2 * ktp + j
                            cs = slice(j * QCHUNK, (j + 1) * QCHUNK)
                            nc.tensor.matmul(
                                sp[:, cs],
                                kT_sb[hp][prow, kt * 128 : (kt + 1) * 128],
                                qT_sb[hp][prow, qs],
                                start=True,
                                stop=False,
                            )
                            nc.tensor.matmul(
                                sp[:, cs], id_sb[:], a_sb[:, kt, :],
                                start=False, stop=True,
                            )
                        nc.scalar.activation(
                            pu[:, 2 * ktp : 2 * ktp + 2, :], sp[:], AF.Exp,
                            scale=0.125,
                        )
                    # ---- outT' = [V|1].T @ Pu ----
                    pv = pv_pool.tile([D + 1, QCHUNK], F32, tag="pv")
                    for kt in range(NKT):
                        nc.tensor.matmul(
                            pv[:], v_sb[h][:, kt, :], pu[:, kt, :],
                            start=(kt == 0), stop=(kt == NKT - 1),
                        )
                    # ---- r = exp(-ln(denom)); rb = broadcast(r) ----
                    lnd = small.tile([D + 1, QCHUNK], F32, tag="lnd")
                    nc.scalar.activation(lnd[D : D + 1, :], pv[D : D + 1, :], AF.Ln)
                    r1 = small.tile([D + 1, QCHUNK], F32, tag="r1")
                    nc.scalar.activation(
                        r1[D : D + 1, :], lnd[D : D + 1, :], AF.Exp, scale=-1.0
                    )
                    bc = bc_pool.tile([128, QCHUNK], F32, tag="bc")
                    nc.tensor.matmul(
                        bc[:], ones_sb[D : D + 1, :], r1[D : D + 1, :],
                        start=True, stop=True,
                    )
                    rb = small.tile([D, QCHUNK], F32, tag="rb")
                    nc.vector.tensor_copy(rb[:], bc[0:D, :])
                    # ---- outT column for this head ----
                    nc.vector.tensor_mul(osb[:, h, :], pv[0:D, :], rb[:])
                    # ---- p_attn^T = Pu * rb, in place; store ----
                    bc_ap = bc[:]
                    bc_rep = bass.AP(
                        tensor=bc_ap.tensor,
                        offset=bc_ap.offset,
                        ap=[bc_ap.ap[0], [0, NKT], bc_ap.ap[1]],
                    )
                    nc.vector.tensor_mul(pu[:], pu[:], bc_rep)
                    nc.sync.dma_start(pT_re[h][:, :, qs], pu[:])
                # one batched outT store per q-chunk: [d, h, q] -> [h, d, q]
                o_ap = outT_d[:]
                o_dst = bass.AP(
                    tensor=o_ap.tensor,
                    offset=o_ap.offset + qc * QCHUNK,
                    ap=[[S, D], [D * S, hpc], [1, QCHUNK]],
                )
                nc.gpsimd.dma_start(o_dst, osb[:])

    nc.compile()
    return nc


_CACHE = {}


def _get_nc():
    if "nc" not in _CACHE:
        _CACHE["nc"] = build()
    return _CACHE["nc"]


def _make_in_maps(query, key, value, mask):
    ident = np.eye(128, dtype=np.float32).astype(ml_dtypes.float8_e5m2)
    in_maps = []
    for c in range(NCORES):
        b, h0 = divmod(c, NCORES // B)
        h0 *= HPC
        q4 = query[b, h0 : h0 + HPC]  # [4, S, D]
        k4 = key[b, h0 : h0 + HPC]
        v4 = np.concatenate(
            [
                np.asarray(value[b, h0 : h0 + HPC], dtype=np.float32),
                np.ones((HPC, S, 1), dtype=np.float32),
            ],
            axis=-1,
        )  # [4, S, 65]
        # [h, S, 65] -> [h, ktile, 128, 65] -> [h, 128, ktile, 65]
        v4 = np.ascontiguousarray(
            v4.reshape(HPC, NKT, 128, D + 1).transpose(0, 2, 1, 3)
        )
        qT = np.ascontiguousarray(q4.transpose(0, 2, 1)).reshape(HPC // 2, 128, S)
        kT = np.ascontiguousarray(k4.transpose(0, 2, 1)).reshape(HPC // 2, 128, S)
        mA = ((mask[b, 0].T.astype(np.float32) - 1.0) * (-AMASK)).astype(
            ml_dtypes.float8_e5m2
        )
        # [k, q] -> [p, qc, ktile, q'] so each partition's per-chunk slice is
        # 8KB contiguous in DRAM
        mA = mA.reshape(NKT, 128, NQC, QCHUNK).transpose(1, 2, 0, 3)
        in_maps.append(
            {"qT": qT, "kT": kT, "v": v4, "mA": np.ascontiguousarray(mA),
             "ident": ident}
        )
    return in_maps


def run_device(query, key, value, mask, trace=False, **kw):
    nc = _get_nc()
    in_maps = _make_in_maps(query, key, value, mask)
    return run_bass_kernel_spmd(nc, in_maps, list(range(NCORES)), trace=trace, **kw)


def kernel(query, key, value, mask):
    query = np.asarray(query, dtype=np.float32)
    key = np.asarray(key, dtype=np.float32)
    value = np.asarray(value, dtype=np.float32)
    mask = np.asarray(mask)
    res = run_device(query, key, value, mask)
    out = np.empty((B, H, S, D), dtype=np.float32)
    p_attn = np.empty((B, H, S, S), dtype=np.float32)
    for c, r in enumerate(res.results):
        b, h0 = divmod(c, NCORES // B)
        h0 *= HPC
        out[b, h0 : h0 + HPC] = r["outT"].transpose(0, 2, 1)
        p_attn[b, h0 : h0 + HPC] = r["pT"].transpose(0, 2, 1)
    return out, p_attn
